# revision 14
# baseline (speedup 1.0000x reference)
"""DIST loss (hard CE + inter/intra Pearson distillation) on 8 Trainium2 cores.

Strategy: data-parallel over the batch dim (4096 rows -> 512 rows/core, 4
row-blocks of 128 partitions each). Per block the core streams z_s/z_t once
from HBM in [128, 4000] f32 tiles (15.6KB/partition DMA lines), computes
exp() on ScalarE (bf16 exponentials cached in SBUF, row sums Zs/Zt free via
the activation accumulator), then per 2000-col chunk produces products +
row sums in fused single passes:
  p11 = es^2   DVE scalar_tensor_tensor (accum U11) / some ScalarE Square
  p22 = et^2   DVE scalar_tensor_tensor (accum U22 free) / ScalarE in tail
  p12 = es*et  DVE scalar_tensor_tensor (accum U12 free)
with the ScalarE/DVE split chosen so both engines run just under the DMA
pace (ScalarE's exps dominate its budget, so DVE carries most products);
the last block (no exps to issue) shifts squares onto ScalarE to shrink
the drain tail.  Per-column weighted partial sums via TensorE: 5
accumulating matmuls per 512-col sub-slice with per-stat weight columns
(1/Zs etc.) as stationary; the 4 sub-slices of a chunk land at PSUM base
partitions 0/32/64 x col offsets 0/512 of a 2-bank tile, so one [69,1024]
copy evacuates the chunk and only the 4x[5,512] used rows are DMA'd out.
Next block's exps are interleaved into the consume loop so every engine
queue stays in feasible dependency order.
The host sums the partial column stats over blocks/cores and finishes the
O(B + C) scalar math (Pearson means, label gather, log) in float64.
"""
import sys
import types
import numpy as np

sys.path.insert(0, "/opt/trn_rl_repo")

B, C = 4096, 32000
N_CORES = 8
R = B // N_CORES          # 512 rows per core
P = 128                   # partitions
NBLK = R // P             # 4 row blocks per core
DTILE = 4000              # DMA/exp tile width
NDT = C // DTILE          # 8 exp tiles per block per tensor
CK = 2000                 # compute chunk width
NCH = C // CK             # 16 chunks per block
SUB = 512                 # PSUM sub-slice width
NSUB = 4                  # sub-slices per chunk: 3x512 + 464
EPS = 1e-8

_built = None


def _install_ntff_shim():
    # antenv.axon_hooks is absent in this image; register the ctypes NTFF
    # hook so run_bass_kernel_spmd(trace=True) can profile under axon.
    try:
        import antenv
        import trn_agent_boot.trn_boot as tb
        if "antenv.axon_hooks" in sys.modules:
            return
        hook = tb._ntff_profile_via_ctypes("/opt/axon/libaxon_pjrt.so")
        mod = types.ModuleType("antenv.axon_hooks")
        mod.get_axon_ntff_profile_hook = lambda: hook
        mod.set_axon_ntff_profile_hook = lambda h: None
        antenv.axon_hooks = mod
        sys.modules["antenv.axon_hooks"] = mod
    except Exception:
        pass


def _sub_w(s):
    return SUB if s < NSUB - 1 else CK - (NSUB - 1) * SUB


def _build():
    from contextlib import ExitStack
    import concourse.bacc as bacc
    import concourse.tile as tile
    from concourse import mybir

    f32 = mybir.dt.float32
    bf16 = mybir.dt.bfloat16
    fp8 = mybir.dt.float8e4
    Exp = mybir.ActivationFunctionType.Exp
    Square = mybir.ActivationFunctionType.Square
    ADD = mybir.AluOpType.add
    MULT = mybir.AluOpType.mult
    AXF = mybir.AxisListType.X

    nc = bacc.Bacc("TRN2", target_bir_lowering=False, debug=False)
    zs_d = nc.dram_tensor("z_s", [R, C], f32, kind="ExternalInput")
    zt_d = nc.dram_tensor("z_t", [R, C], f32, kind="ExternalInput")
    col_d = nc.dram_tensor("colstats", [NBLK, NCH, NSUB, 5, SUB], bf16,
                           kind="ExternalOutput")
    row_d = nc.dram_tensor("rowstats", [R, 8], f32, kind="ExternalOutput")

    with tile.TileContext(nc) as tc, ExitStack() as ctx:
        zin = ctx.enter_context(tc.tile_pool(name="zin", bufs=5))
        esp = ctx.enter_context(tc.tile_pool(name="esp", bufs=NDT + 4))
        etp = ctx.enter_context(tc.tile_pool(name="etp", bufs=NDT + 4))
        prod = ctx.enter_context(tc.tile_pool(name="prod", bufs=6))
        statp = ctx.enter_context(tc.tile_pool(name="stat", bufs=2))
        small = ctx.enter_context(tc.tile_pool(name="small", bufs=2))
        psump = ctx.enter_context(tc.tile_pool(name="psum", bufs=4, space="PSUM"))

        es_tiles = [None] * NDT
        et_tiles = [None] * NDT
        state = {}

        def emit_exp(b, d):
            r0 = b * P
            c0 = d * DTILE
            if d == 0:
                state["zsp"] = small.tile([P, NDT], f32, tag="zsp", name="zsp")
                state["ztp"] = small.tile([P, NDT], f32, tag="ztp", name="ztp")
            zs = zin.tile([P, DTILE], f32, tag="zin")
            nc.sync.dma_start(zs[:], zs_d[r0:r0 + P, c0:c0 + DTILE])
            es = esp.tile([P, DTILE], fp8, tag="es")
            nc.scalar.activation(es[:], zs[:], Exp,
                                 accum_out=state["zsp"][:, d:d + 1])
            zt = zin.tile([P, DTILE], f32, tag="zin")
            nc.sync.dma_start(zt[:], zt_d[r0:r0 + P, c0:c0 + DTILE])
            et = etp.tile([P, DTILE], fp8, tag="et")
            nc.scalar.activation(et[:], zt[:], Exp,
                                 accum_out=state["ztp"][:, d:d + 1])
            es_tiles[d] = es
            et_tiles[d] = et

        def emit_wprep(b):
            rst = small.tile([P, 8], f32, tag="rst")
            nc.vector.tensor_reduce(rst[:, 0:1], state["zsp"][:, 0:NDT],
                                    axis=AXF, op=ADD)
            nc.vector.tensor_reduce(rst[:, 1:2], state["ztp"][:, 0:NDT],
                                    axis=AXF, op=ADD)
            w1 = small.tile([P, 1], f32, tag="w1")
            nc.vector.reciprocal(w1[:], rst[:, 0:1])
            w2 = small.tile([P, 1], f32, tag="w2")
            nc.vector.reciprocal(w2[:], rst[:, 1:2])
            W = []
            for k in range(5):
                Wk = small.tile([P, 5], bf16, tag=f"W{k}")
                nc.vector.memset(Wk[:], 0.0)
                W.append(Wk)
            nc.vector.tensor_copy(W[0][:, 0:1], w1[:])
            nc.vector.tensor_copy(W[1][:, 1:2], w2[:])
            nc.vector.tensor_mul(W[2][:, 2:3], w1[:], w1[:])
            nc.vector.tensor_mul(W[3][:, 3:4], w2[:], w2[:])
            nc.vector.tensor_mul(W[4][:, 4:5], w1[:], w2[:])
            state["W"] = W
            state["rst"] = rst
            state["u11p"] = small.tile([P, NCH], f32, tag="u11p", name="u11p")
            state["u22p"] = small.tile([P, NCH], f32, tag="u22p", name="u22p")
            state["u12p"] = small.tile([P, NCH], f32, tag="u12p", name="u12p")

        def emit_consume(b, ci):
            d, half = divmod(ci, 2)
            off = half * CK
            esv = es_tiles[d][:, off:off + CK]
            etv = et_tiles[d][:, off:off + CK]
            u11p, u22p, u12p = state["u11p"], state["u22p"], state["u12p"]
            last = b == NBLK - 1
            # Engine split: ScalarE's exps for the next block dominate its
            # budget, so DVE carries most product passes; the exp-free last
            # block leans on ScalarE instead to shrink the drain tail.
            if last:
                p11_scalar = True
                p22_scalar = ci % 2 == 0
                evac_scalar = ci % 2 == 0
            else:
                p11_scalar = ci % 2 == 0
                p22_scalar = False
                evac_scalar = ci % 2 == 1

            def emit_prod(src0, src1, acc, on_scalar):
                pt = prod.tile([P, CK], bf16, tag="prod", name="pt")
                if on_scalar:
                    nc.scalar.activation(pt[:], src0, Square, accum_out=acc)
                else:
                    nc.vector.scalar_tensor_tensor(pt[:], src0, 1.0, src1,
                                                   MULT, MULT, accum_out=acc)
                return pt

            p11 = emit_prod(esv, esv, u11p[:, ci:ci + 1], p11_scalar)
            p22 = emit_prod(etv, etv, u22p[:, ci:ci + 1], p22_scalar)
            p12 = prod.tile([P, CK], bf16, tag="prod")
            nc.vector.scalar_tensor_tensor(p12[:], esv, 1.0, etv, MULT, MULT,
                                           accum_out=u12p[:, ci:ci + 1])
            rhs = [esv, etv, p11, p22, p12]
            W = state["W"]
            # PE out base partition must be 0/32/64: sub-slice s lands at
            # partition 32*(s%3), col offset 512*(s//3) of a 2-bank tile.
            ps = psump.tile([69, 2 * SUB], f32, tag="ps")
            for s in range(NSUB):
                w = _sub_w(s)
                p0 = 32 * (s % 3)
                o0 = SUB * (s // 3)
                for k in range(5):
                    nc.tensor.matmul(ps[p0:p0 + 5, o0:o0 + w],
                                     W[k][:, 0:5],
                                     rhs[k][:, s * SUB:s * SUB + w],
                                     start=(k == 0), stop=(k == 4))
            st = statp.tile([69, 2 * SUB], bf16, tag="st")
            if evac_scalar:
                nc.scalar.copy(st[:], ps[:])
            else:
                nc.vector.tensor_copy(st[:], ps[:])
            for s in range(NSUB):
                p0 = 32 * (s % 3)
                o0 = SUB * (s // 3)
                nc.sync.dma_start(col_d[b, ci, s],
                                  st[p0:p0 + 5, o0:o0 + SUB])

        def emit_rowfin(b):
            r0 = b * P
            rst = state["rst"]
            nc.vector.tensor_reduce(rst[:, 2:3], state["u11p"][:, 0:NCH],
                                    axis=AXF, op=ADD)
            nc.vector.tensor_reduce(rst[:, 3:4], state["u22p"][:, 0:NCH],
                                    axis=AXF, op=ADD)
            nc.vector.tensor_reduce(rst[:, 4:5], state["u12p"][:, 0:NCH],
                                    axis=AXF, op=ADD)
            nc.sync.dma_start(row_d[r0:r0 + P, 0:5], rst[:, 0:5])

        for d in range(NDT):
            emit_exp(0, d)
        for b in range(NBLK):
            emit_wprep(b)
            for ci in range(NCH):
                emit_consume(b, ci)
                if b + 1 < NBLK and ci < NDT:
                    emit_exp(b + 1, ci)
            emit_rowfin(b)

    nc.compile()
    return nc


def _get_built():
    global _built
    if _built is None:
        _install_ntff_shim()
        _built = _build()
    return _built


def _unpack_col(colstats):
    """colstats [NBLK, NCH, NSUB, 5, SUB] f32 -> [5, C] float64 column stats."""
    acc = colstats.astype(np.float64).sum(axis=0)   # [NCH, NSUB, 5, SUB]
    col = np.zeros((5, C), np.float64)
    for ci in range(NCH):
        for s in range(NSUB):
            w = _sub_w(s)
            c0 = ci * CK + s * SUB
            col[:, c0:c0 + w] += acc[ci, s][:, 0:w]
    return col


def run_sharded(z_s, z_t, trace=False, tmpdir=None):
    """Run the device program; returns (colstats_sum [5, C] f64,
    rowstats [B, 5] f64, BassKernelResults)."""
    from concourse.bass_utils import run_bass_kernel_spmd

    nc = _get_built()
    z_s = np.ascontiguousarray(np.asarray(z_s, dtype=np.float32))
    z_t = np.ascontiguousarray(np.asarray(z_t, dtype=np.float32))
    in_maps = [
        {"z_s": z_s[i * R:(i + 1) * R], "z_t": z_t[i * R:(i + 1) * R]}
        for i in range(N_CORES)
    ]
    res = run_bass_kernel_spmd(nc, in_maps, core_ids=list(range(N_CORES)),
                               trace=trace, tmpdir=tmpdir)
    col = np.zeros((5, C), np.float64)
    rows = []
    for i in range(N_CORES):
        col += _unpack_col(res.results[i]["colstats"])
        rows.append(res.results[i]["rowstats"][:, :5].astype(np.float64))
    return col, np.concatenate(rows, axis=0), res


def kernel(z_s, z_t, labels):
    col, rowstats, _ = run_sharded(z_s, z_t)
    return _finish(np.asarray(z_s), np.asarray(labels), col, rowstats)


def _finish(z_s, labels, col, rowstats):
    Zs, Zt, U11, U22, U12 = rowstats.T
    invC = 1.0 / C
    # inter: Pearson over classes per row (softmax rows have mean 1/C)
    num = U12 / (Zs * Zt) - invC
    vs = U11 / (Zs * Zs) - invC
    vt = U22 / (Zt * Zt) - invC
    corr = num / (np.sqrt(vs) * np.sqrt(vt) + EPS)
    inter = 1.0 - corr.mean()
    # intra: Pearson over samples per column
    S1, S2, S11, S22, S12 = col
    numc = S12 - S1 * S2 / B
    vsc = S11 - S1 * S1 / B
    vtc = S22 - S2 * S2 / B
    corrc = numc / (np.sqrt(vsc) * np.sqrt(vtc) + EPS)
    intra = 1.0 - corrc.mean()
    # hard CE: mean(logsumexp(z_s) - z_s[label])
    lab = np.asarray(labels).astype(np.int64).ravel()
    zl = z_s[np.arange(B), lab].astype(np.float64)
    hard = (np.log(Zs) - zl).mean()
    return np.float32(hard + inter + intra)
